# revision 46
# baseline (speedup 1.0000x reference)
"""Trainium2 Bass kernel for AccentVarianceAdaptor (v7: step-matrix telescoping).

Computation (per batch row):
  pbin = searchsorted(linspace(50,400,256), clip(pitch,50,400), 'left')
  ebin = searchsorted(linspace(0,1,256),  clip(energy,0,1),  'left')
  y    = encoder + ptab[pbin] + etab[ebin]               # [S, H]
  dur  = max(round(duration), 1); cum = cumsum(dur)
  out[t] = y[searchsorted(cum, t, 'right')] * (t < cum[-1])  # [T, H]

v7 design (one NeuronCore handles 4 batch rows, everything on-chip):
  - Telescoping over TOKENS: out[f] = sum_j step[j,f]*dy[j] where
    step[j,f] = (f < cum[j]) and dy[j] = y[j] - y[j+1] (dy[511]=y[511]).
    The sum telescopes to y[idx(f)] exactly, and frames past cum[511]
    get 0 for free. No frame-index scatter/scan/broadcast machinery.
  - step built by ONE DVE is_lt per token chunk against a constant
    int16 frame-id tile, scalar = per-partition cum column (int16,
    exact to 4096). Statically-zero (token chunks fully below the
    frame window) are skipped; chunks fully above contribute all-ones
    columns -> cheap all-ones matmuls (colsum of dy chunk).
  - dy = 4 bf16 matmuls per chunk with DIFFERENCED compare matrices
    dcp[b,j] = cp[b,j]-cp[b,j+1] (cp[b,j] = boundary[b] < v[j]), plus
    denc[j] = enc'[j]-enc'[j+1] precomputed on host (enc' folds the
    two base table rows), added on DVE. PSUM f32 accumulate, dy f16.
  - durations: (d + 2^23) - 2^23 rounds half-to-even exactly in f32;
    per-chunk tensor_tensor_scan + tiny matmuls for cross-chunk offsets.
  - outputs stored f16 (values are exact f16 already); host upcasts.
  - error budget: fp16 quantization of dy telescopes over <=512 terms
    -> rel_err ~ 7e-3 (measured in numpy), well under the 2e-2 gate.
"""

import os
import sys

for _p in ("/opt/trn_rl_repo", "/root/.axon_site/_ro/trn_rl_repo"):
    if os.path.isdir(_p) and _p not in sys.path:
        sys.path.insert(0, _p)

import numpy as np

from concourse import bacc, mybir, tile
from concourse.bass import AP, ts
from concourse.bass_utils import run_bass_kernel_spmd

B, S, H = 32, 512, 256
NBINS = 256
T = 4096
NCORES = 8
BPC = B // NCORES  # batches per core
P = 128
NCH = S // P  # token chunks per batch (4)
NFT = T // P  # frame tiles (32)
F32 = mybir.dt.float32
F16 = mybir.dt.float16
BF16 = mybir.dt.bfloat16
I16 = mybir.dt.int16
A = mybir.AluOpType
AF = mybir.ActivationFunctionType

# (name, rows, cols) layout of the packed f32 constant block
_PACKF = [("bndp", P, 2), ("bnde", P, 2)]
PACKF_COLS = sum(c for _, _, c in _PACKF)

# frame-tile coverage of token chunk k: fts k .. min(8k+7, 31)
_FTLO = {k: k for k in range(NCH)}
_FTHI = {k: min(8 * k + 7, NFT - 1) for k in range(NCH)}


def _var_chunks(ft):
    """Token chunks whose step column varies within frame tile ft."""
    return range(ft // 8, min(NCH - 1, ft) + 1)


def _ones_chunks(ft):
    """Token chunks fully above frame tile ft (step identically 1)."""
    return range(min(NCH - 1, ft) + 1, NCH)


def _boundaries():
    """Bit-exact copies of the f32 boundaries the jax reference uses."""
    import jax

    with jax.default_device(jax.devices("cpu")[0]):
        import jax.numpy as jnp

        bp = np.asarray(jnp.linspace(50.0, 400.0, NBINS), np.float32)
        be = np.asarray(jnp.linspace(0.0, 1.0, NBINS), np.float32)
    return bp, be


def _host_constants(pitch_table, energy_table):
    bp, be = _boundaries()
    consts = {}
    import ml_dtypes
    tabs = np.zeros((2, NBINS, H), np.float32)
    for i, tab in enumerate((pitch_table, energy_table)):
        tabs[i, :-1] = tab[1:] - tab[:-1]  # f32 arithmetic, row 255 stays 0
    consts["dtabs"] = tabs.reshape(2 * NBINS, H).astype(ml_dtypes.bfloat16)
    # boundaries rescaled to the same f16-friendly domain as the values
    bps = (bp - 225.0) / 175.0
    bes = (be - 0.5) * 2.0
    valsf = {
        "bndp": bps.reshape(2, P).T.copy(),  # [128, 2], col h = b[h*128 + p]
        "bnde": bes.reshape(2, P).T.copy(),
    }
    pk = np.zeros((P, PACKF_COLS), np.float32)
    c0 = 0
    for name, rows, cols in _PACKF:
        pk[:rows, c0 : c0 + cols] = valsf[name]
        c0 += cols
    consts["pconstf"] = pk
    consts["frameid"] = np.broadcast_to(
        np.arange(T, dtype=np.int16)[None, :], (P, T)
    ).copy()
    return consts


def build_nc():
    nc = bacc.Bacc("TRN2", target_bir_lowering=False, debug=False, enable_asserts=False)

    denc_dr = nc.dram_tensor("denc", [BPC, S, H], F16, kind="ExternalInput")
    pit_dr = nc.dram_tensor("pitch", [BPC, S], F16, kind="ExternalInput")
    ene_dr = nc.dram_tensor("energy", [BPC, S], F16, kind="ExternalInput")
    dur_dr = nc.dram_tensor("durt", [BPC, S], F32, kind="ExternalInput")
    tabs_dr = nc.dram_tensor("dtabs", [2 * NBINS, H], BF16, kind="ExternalInput")
    pkf_dr = nc.dram_tensor("pconstf", [P, PACKF_COLS], F32, kind="ExternalInput")
    fid_dr = nc.dram_tensor("frameid", [P, T], I16, kind="ExternalInput")
    out_dr = [
        nc.dram_tensor(f"out{b}", [T, H], F16, kind="ExternalOutput")
        for b in range(BPC)
    ]

    with tile.TileContext(nc) as tc:
        with (
            tc.tile_pool(name="const", bufs=1) as cp,
            tc.tile_pool(name="work", bufs=2) as wp,
            tc.tile_pool(name="inb", bufs=1) as ib,
            tc.tile_pool(name="dytil", bufs=3) as yp,
            tc.tile_pool(name="stept", bufs=2) as sp,
            tc.tile_pool(name="gat", bufs=3) as gp,
            tc.tile_pool(name="pout", bufs=3, space="PSUM") as po,
            tc.tile_pool(name="peps", bufs=2, space="PSUM") as psm,
        ):
            # ---- constants + inputs; row-0 dependencies land first ----
            # tiny f16 constants are generated on-chip so the first matmuls
            # don't wait on any DMA-completion semaphore
            csb = {}
            m128 = cp.tile([P, P], F32, tag="m128")
            nc.gpsimd.iota(out=m128[:], pattern=[[1, P]], base=0, channel_multiplier=-1, allow_small_or_imprecise_dtypes=True)
            ule = cp.tile([P, P], F16, tag="ule")
            nc.vector.tensor_scalar(out=ule[:], in0=m128[:], scalar1=0.0, scalar2=None, op0=A.is_ge)
            id16 = cp.tile([BPC * NCH, BPC * NCH], F16, tag="id16")
            nc.vector.tensor_scalar(out=id16[:], in0=m128[0 : BPC * NCH, 0 : BPC * NCH], scalar1=0.0, scalar2=None, op0=A.is_equal)
            allones = cp.tile([P, P], F16, tag="allones")
            nc.gpsimd.memset(allones[:], 1.0)
            csb.update(ule=ule, allones=allones, id16=id16)
            pkf_sb = cp.tile([P, PACKF_COLS], F32, tag="pconstf")
            nc.sync.dma_start(out=pkf_sb[:], in_=pkf_dr[:])
            c0 = 0
            for name, rows, cols in _PACKF:
                csb[name] = pkf_sb[0:rows, c0 : c0 + cols]
                c0 += cols
            # durations first (they gate the cum path), then row-0 values,
            # then everything else (HWDGE completes serially per queue,
            # ~2-4us per DMA including the receipt latency).
            dur_all = ib.tile([BPC * NCH, P], F32, tag="dur_all")
            nc.sync.dma_start(
                out=dur_all[:], in_=dur_dr[:].rearrange("b (c p) -> (b c) p", p=P)
            )

            vp_all = ib.tile([P, BPC, S], F16, tag="vp_all")
            ve_all = ib.tile([P, BPC, S], F16, tag="ve_all")
            nc.scalar.dma_start(
                out=vp_all[:, 0, :], in_=pit_dr[0][None, :].to_broadcast([P, S])
            )
            nc.scalar.dma_start(
                out=ve_all[:, 0, :], in_=ene_dr[0][None, :].to_broadcast([P, S])
            )
            fid_sb = cp.tile([P, T], I16, tag="frameid")
            nc.scalar.dma_start(out=fid_sb[:], in_=fid_dr[:])
            tabs_sb = cp.tile([P, 2, 2, H], BF16, tag="dtabs")
            nc.sync.dma_start(
                out=tabs_sb[:], in_=tabs_dr[:].rearrange("(t h p) f -> p t h f", t=2, h=2, p=P)
            )
            csb["dpt_hi"] = tabs_sb[:, 0]
            csb["det_hi"] = tabs_sb[:, 1]
            nc.sync.dma_start(
                out=vp_all[:, 1:BPC, :],
                in_=pit_dr[1:BPC].rearrange("b s -> (b s)")[None, :].to_broadcast(
                    [P, (BPC - 1) * S]
                ),
            )
            nc.scalar.dma_start(
                out=ve_all[:, 1:BPC, :],
                in_=ene_dr[1:BPC].rearrange("b s -> (b s)")[None, :].to_broadcast(
                    [P, (BPC - 1) * S]
                ),
            )
            vp_reps = {b: vp_all[:, b, :] for b in range(BPC)}
            ve_reps = {b: ve_all[:, b, :] for b in range(BPC)}

            denc_all = ib.tile([P, BPC, NCH, H], F16, tag="denc_all")
            nc.sync.dma_start(
                out=denc_all[:, 0],
                in_=denc_dr[0].rearrange("(c p) f -> p c f", p=P),
            )
            nc.sync.dma_start(
                out=denc_all[:, 1:BPC],
                in_=denc_dr[1:BPC].rearrange("b (c p) f -> p b c f", p=P),
            )
            denc_sb = {b: denc_all[:, b] for b in range(BPC)}

            cum16s = {}
            dcp_tiles = {}
            dy_tiles = {}
            step_tiles = {}

            # ---- dur = max(round_half_even(durt), 1), all rows at once,
            # in the contiguous [(b c), p] layout, then transpose by matmul
            MAGIC = float(1 << 23)
            dr0_all = ib.tile([BPC * NCH, P], F32, tag="dr0_all")
            nc.vector.tensor_scalar(out=dr0_all[:], in0=dur_all[:], scalar1=MAGIC, scalar2=MAGIC, op0=A.add, op1=A.subtract)
            dur4_t = ib.tile([BPC * NCH, P], F16, tag="dur4_t")
            nc.vector.tensor_scalar(out=dur4_t[:], in0=dr0_all[:], scalar1=1.0, scalar2=None, op0=A.max)

            def phase_cum_all():
                # transpose dur to [p, (b c)] via one matmul
                durT_big = po.tile([P, 1024], F32, tag="out")
                durT_ps = durT_big[:, 0 : BPC * NCH]
                nc.tensor.matmul(out=durT_ps[:], lhsT=dur4_t[:], rhs=csb["id16"][:], start=True, stop=True)
                dur4_all = ib.tile([P, BPC, NCH], F16, tag="dur4_all")
                nc.vector.tensor_copy(out=dur4_all[:], in_=durT_ps[:])
                # ---- cum[p, b, c] = sum_{(c',q) <= (c,p)} dur[q, b, c'] via
                # 10 tiny matmuls (triangular / all-ones lhsT), all rows at once
                cum_big = po.tile([P, 1024], F32, tag="out")
                cum_ps = cum_big[:, 0 : BPC * NCH]
                for c in range(NCH):
                    for c2 in range(c + 1):
                        lhsT_ = csb["ule"][:] if c2 == c else csb["allones"][:]
                        nc.tensor.matmul(
                            out=cum_ps[:, c * BPC : (c + 1) * BPC],
                            lhsT=lhsT_,
                            rhs=dur4_all[:, :, c2],
                            start=(c2 == 0), stop=(c2 == c),
                        )
                cumf = wp.tile([P, NCH, BPC], F32, tag="cumf")
                nc.vector.tensor_copy(out=cumf[:], in_=cum_ps[:])
                for b in range(BPC):
                    cum16s[b] = cumf[:, :, b]

            def phase_cmp(b):
                # ---- dcp[bin, j] = cp[bin, j] - cp[bin, j+1] (bf16) ----
                # cp tiles have a zero col at 512 so the shifted subtract
                # covers j=511 (dcp[:,511] = cp[:,511]).
                srcs = (
                    ("p0", vp_reps[b], csb["bndp"][:, 0:1], nc.gpsimd),
                    ("e0", ve_reps[b], csb["bnde"][:, 0:1], nc.gpsimd),
                    ("p1", vp_reps[b], csb["bndp"][:, 1:2], nc.vector),
                    ("e1", ve_reps[b], csb["bnde"][:, 1:2], nc.vector),
                )
                for nm, v_, bnd, dcp_eng in srcs:
                    ct = wp.tile([P, S + 1], BF16, tag=f"cp{nm}")
                    nc.gpsimd.memset(ct[:, S : S + 1], 0.0)
                    nc.vector.tensor_scalar(out=ct[:, 0:S], in0=v_[:], scalar1=bnd, scalar2=None, op0=A.is_gt)
                    dt_ = wp.tile([P, S], BF16, tag=f"dcp{nm}")
                    dcp_eng.tensor_tensor(out=dt_[:], in0=ct[:, 0:S], in1=ct[:, 1 : S + 1], op=A.subtract)
                    dcp_tiles[(b, nm)] = dt_

            def phase_step_one(b, c):
                # step[tok_p, f] = (f < cum[tok]) for chunk c, f16
                span = (_FTHI[c] - _FTLO[c] + 1) * P
                st = sp.tile([P, span], F16, tag=f"st{c}")
                nc.vector.tensor_scalar(
                    out=st[:], in0=fid_sb[:, _FTLO[c] * P : (_FTHI[c] + 1) * P],
                    scalar1=cum16s[b][:, c : c + 1], scalar2=None,
                    op0=A.is_lt,
                )
                step_tiles[(b, c)] = st

            def phase_dy(b, with_steps=False):
                # ---- dy = dcp.T @ dtabs (+ denc on DVE), f16 ----
                dy_sb = yp.tile([P, NCH, H], F16, tag="dy")
                for c in range(NCH):
                    if c % 2 == 0:
                        eps2 = psm.tile([P, 2, H], F32, tag="eps")
                    eps = eps2[:, c % 2, :]
                    nc.tensor.matmul(out=eps[:], lhsT=dcp_tiles[(b, "p0")][:, ts(c, P)], rhs=csb["dpt_hi"][:, 0, :], start=True, stop=False)
                    nc.tensor.matmul(out=eps[:], lhsT=dcp_tiles[(b, "p1")][:, ts(c, P)], rhs=csb["dpt_hi"][:, 1, :], start=False, stop=False)
                    nc.tensor.matmul(out=eps[:], lhsT=dcp_tiles[(b, "e0")][:, ts(c, P)], rhs=csb["det_hi"][:, 0, :], start=False, stop=False)
                    nc.tensor.matmul(out=eps[:], lhsT=dcp_tiles[(b, "e1")][:, ts(c, P)], rhs=csb["det_hi"][:, 1, :], start=False, stop=True)
                    nc.vector.tensor_tensor(out=dy_sb[:, c, :], in0=eps[:], in1=denc_sb[b][:, c, :], op=A.add)
                    if with_steps:
                        phase_step_one(b, c)
                dy_tiles[b] = dy_sb

            def phase_out(b, g8):
                # ---- out[ft*128+p, :] = sum_k step_k.T @ dy_k ----
                dy_sb = dy_tiles[b]
                gbuf = gp.tile([P, 16, H], F16, tag="g")
                for q in range(4):  # quads of frame tiles
                    out_ps = po.tile([P, 1024], F32, tag="out")
                    for half in range(4):
                        ft = g8 * 16 + q * 4 + half
                        mms = [
                            (step_tiles[(b, k)][:, (ft - _FTLO[k]) * P : (ft - _FTLO[k] + 1) * P], dy_sb[:, k, :])
                            for k in _var_chunks(ft)
                        ] + [
                            (csb["allones"][:], dy_sb[:, k, :])
                            for k in _ones_chunks(ft)
                        ]
                        for j, (lhsT_, rhs_) in enumerate(mms):
                            nc.tensor.matmul(
                                out=out_ps[:, half * H : (half + 1) * H],
                                lhsT=lhsT_, rhs=rhs_,
                                start=(j == 0), stop=(j == len(mms) - 1),
                            )
                    # PSUM -> SBUF f16: ACT mostly; DVE helps drain the tail
                    if b == BPC - 1:
                        nc.scalar.activation(
                            out=gbuf[:, 4 * q : 4 * q + 2, :], in_=out_ps[:, 0:512], func=AF.Copy
                        )
                        nc.vector.tensor_copy(
                            out=gbuf[:, 4 * q + 2 : 4 * q + 4, :], in_=out_ps[:, 512:1024]
                        )
                    else:
                        nc.scalar.activation(
                            out=gbuf[:, 4 * q : 4 * q + 4, :], in_=out_ps[:], func=AF.Copy
                        )
                    if b == BPC - 1:
                        # tail trim: store each quad as soon as it is copied
                        eng = nc.sync if q % 2 == 0 else nc.scalar
                        f0 = g8 * 2048 + q * 512
                        eng.dma_start(
                            out=out_dr[b][f0 : f0 + 512, :].rearrange(
                                "(c p) f -> p c f", p=P
                            ),
                            in_=gbuf[:, 4 * q : 4 * q + 4, :],
                        )
                if b != BPC - 1:
                    nc.sync.dma_start(
                        out=out_dr[b][g8 * 2048 : (g8 + 1) * 2048, :].rearrange(
                            "(c p) f -> p c f", p=P
                        ),
                        in_=gbuf[:],
                    )

            phase_cum_all()
            phase_cmp(0)
            phase_dy(0, with_steps=True)
            phase_cmp(1)
            for c in range(NCH):
                phase_step_one(1, c)
            phase_out(0, 0)
            phase_dy(1)
            phase_out(0, 1)
            phase_cmp(2)
            for c in range(NCH):
                phase_step_one(2, c)
            phase_out(1, 0)
            phase_dy(2)
            phase_out(1, 1)
            phase_cmp(3)
            for c in range(NCH):
                phase_step_one(3, c)
            phase_out(2, 0)
            phase_dy(3)
            phase_out(2, 1)
            phase_out(3, 0)
            phase_out(3, 1)

    nc.compile()
    return nc


_NC_CACHE = {}


def _get_nc():
    if "nc" not in _NC_CACHE:
        _NC_CACHE["nc"] = build_nc()
    return _NC_CACHE["nc"]


def make_in_maps(inputs):
    enc = np.asarray(inputs["encoder_output"], np.float32)
    pit32 = np.asarray(inputs["pitch_target"], np.float32)
    ene32 = np.asarray(inputs["energy_target"], np.float32)
    # rescale into an f16-exact-enough domain (boundaries rescaled likewise)
    pit = np.ascontiguousarray(
        ((np.clip(pit32, 50.0, 400.0) - 225.0) / 175.0).astype(np.float16)
    )
    ene = np.ascontiguousarray(
        ((np.clip(ene32, 0.0, 1.0) - 0.5) * 2.0).astype(np.float16)
    )
    dur = np.ascontiguousarray(np.asarray(inputs["duration_target"], np.float32))
    ptab = np.asarray(inputs["pitch_table"], np.float32)
    etab = np.asarray(inputs["energy_table"], np.float32)
    # fold the base table rows into enc, difference along tokens (f32), f16
    encp = enc + (ptab[0] + etab[0])[None, None, :]
    denc = np.empty_like(encp)
    denc[:, :-1] = encp[:, :-1] - encp[:, 1:]
    denc[:, -1] = encp[:, -1]
    denc = np.ascontiguousarray(denc.astype(np.float16))
    consts = _host_constants(ptab, etab)
    in_maps = []
    for c in range(NCORES):
        sl = slice(c * BPC, (c + 1) * BPC)
        m = dict(consts)
        m["denc"] = denc[sl]
        m["pitch"] = pit[sl]
        m["energy"] = ene[sl]
        m["durt"] = dur[sl]
        in_maps.append(m)
    return in_maps


def run(inputs, trace=False):
    nc = _get_nc()
    in_maps = make_in_maps(inputs)
    res = run_bass_kernel_spmd(nc, in_maps, list(range(NCORES)), trace=trace)
    out = np.empty((B, T, H), np.float32)
    for c in range(NCORES):
        for b in range(BPC):
            out[c * BPC + b] = res.results[c][f"out{b}"].astype(np.float32)
    return out, res


def kernel(**inputs):
    out, _ = run(inputs, trace=False)
    return out
